# revision 20
# baseline (speedup 1.0000x reference)
"""CLUB loss kernel for 8x TRN2 NeuronCores.

Math: per sample b (L=512 positions, D=64 dims):
  mu     = MLP_mu(x);  logvar = tanh(MLP_lv(x));  iv = exp(-logvar)
  loss = mean over (b,l) of sum_d (positive - negative)
The pairwise LxL mean collapses via moments of y:
  pos - neg = -(0.5*iv) * (y^2 - Ey2 - mu*yd2),  yd2 = 2*(y - Ey)
so per core:  acc = sum_{d,l} (A - mu*yd2) * iv,  A = y^2 - Ey2,
and the host computes  loss = -0.5 * sum_b acc_b / (B*L).

Schedule (v4, built from perfetto traces):
 - ~13.2us of the measured window is fixed framework overhead (bass
   preamble + NRT's load-time epilogue that resets all 253 semaphores
   one EVENT_SEMAPHORE at a time + barrier butterflies). Body work sits
   on top of that.
 - input DMA placement: wi alone on the scalar HWDGE ring (sharing it
   serialized the completion sem ~2.3us late in v3); xa -> yb -> b1 on
   the sync HWDGE ring; xb on SWDGE (first in queue). HWDGE data is
   consumable ~3.2us after desc-gen (SDMA start + transfer + receipt).
 - PE HAM warmup: 6 K=1 matmuls on a memset [1,640] tile keep the PE
   busy from queue-start until the real matmuls begin, with ~zero SBUF
   read traffic (v2's K=128 dummies at ~450GB/s starved the input-DMA
   writes). HAM flips 1.2->2.4GHz after ~3.4us of sustained activity.
 - ACT runs relu_lv, relu_mu, tanh, exp (full-L each, (N+352)/1.2 ns);
   DVE runs the y-moments and the m2 -> m3 -> f tail; y and all
   elementwise intermediates are fp16 for the DVE 2x perf mode.
 - tail: f accumulates sum_l m3*iv per d; a [64,1]x[64,1] matmul
   collapses d; 4-byte single-packet store.

Precision: matmul operands bf16 (fp32 PE mode is 4x slower and fp32r
truncates to ~bf16 anyway); y/intermediates fp16 (loss error vs fp32
reference measured ~3e-4 with fp32, fp16 adds ~5e-4-level noise, well
under the 2e-2 gate); biases and accumulators fp32.
"""

import sys

if "/opt/trn_rl_repo" not in sys.path:
    sys.path.insert(0, "/opt/trn_rl_repo")

import numpy as np

B, L = 8, 512
XD, YD, H = 192, 64, 128
NCORES = 8
WIC = 640  # w1 pack (512) + w2 pack (128)
NDUMMY = 6  # K=1 N=512 HAM-warmup matmuls, ~427ns each cold

_CACHE: dict = {}


def build_nc(debug: bool = False):
    import concourse.bass as bass
    import concourse.bacc as bacc
    import concourse.tile as tile
    from concourse import mybir
    from concourse.tile import add_dep_helper

    f32 = mybir.dt.float32
    f16 = mybir.dt.float16
    bf16 = mybir.dt.bfloat16
    AF = mybir.ActivationFunctionType
    OP = mybir.AluOpType
    AX = mybir.AxisListType

    nc = bacc.Bacc("TRN2", target_bir_lowering=False, debug=debug)

    wi_d = nc.dram_tensor("wi", [128, WIC], bf16, kind="ExternalInput")
    xa_d = nc.dram_tensor("xa", [128, L], bf16, kind="ExternalInput")
    xb_d = nc.dram_tensor("xb", [64, L], bf16, kind="ExternalInput")
    yb_d = nc.dram_tensor("yb", [64, L], f16, kind="ExternalInput")
    b1_d = nc.dram_tensor("b1", [128, 4], f32, kind="ExternalInput")
    acc_d = nc.dram_tensor("acc", [1, 1], f32, kind="ExternalOutput")

    with tile.TileContext(nc) as tc:
        with (
            tc.tile_pool(name="sb", bufs=1) as sb,
            tc.tile_pool(name="ps", bufs=1, space=bass.MemorySpace.PSUM) as ps,
        ):
            # --- PE HAM warmup: K=1 matmuls on memset data (no SBUF load)
            warm = sb.tile([1, L], bf16, tag="warm")
            mset_warm = nc.gpsimd.memset(warm, 0.0)
            ones = sb.tile([64, 1], f32, tag="ones")
            nc.gpsimd.memset(ones, 1.0)
            dummy_ps = ps.tile([128, L], f32, tag="dummy")
            pe_order = []
            for i in range(NDUMMY):
                pe_order.append(
                    nc.tensor.matmul(
                        dummy_ps, warm[0:1, 0:128], warm[0:1, 0:512],
                        start=True, stop=True,
                    )
                )

            # --- input DMAs ---
            xat = sb.tile([128, L], bf16, tag="xat")
            d_xa = nc.sync.dma_start(out=xat, in_=xa_d[:, :])
            ybt = sb.tile([64, L], f16, tag="ybt")
            d_yb = nc.sync.dma_start(out=ybt, in_=yb_d[:, :])
            b1t = sb.tile([128, 4], f32, tag="b1t")
            d_b1 = nc.sync.dma_start(out=b1t, in_=b1_d[:, :])
            add_dep_helper(d_yb.ins, d_xa.ins, sync=False, reason="ring-order")
            add_dep_helper(d_b1.ins, d_yb.ins, sync=False, reason="ring-order")
            wit = sb.tile([128, WIC], bf16, tag="wit")
            d_wi = nc.scalar.dma_start(out=wit, in_=wi_d[:, :])
            xbt = sb.tile([64, L], bf16, tag="xbt")
            d_xb = nc.scalar.dma_start(out=xbt, in_=xb_d[:, :])
            add_dep_helper(d_xb.ins, d_wi.ins, sync=False, reason="ring-order")

            w1lvT_a = wit[:, 0:128]
            w1muT_a = wit[:, 128:256]
            w1lvT_b = wit[0:64, 256:384]
            w1muT_b = wit[0:64, 384:512]
            w2lvT = wit[:, 512:576]
            w2muT = wit[:, 576:640]
            b1mu = b1t[:, 0:1]
            b1lv = b1t[:, 1:2]
            b2mu = b1t[0:64, 2:3]
            b2lv = b1t[0:64, 3:4]
            xa = xat[:, :]
            xb = xbt[:, :]
            y = ybt[:, :]

            # --- y moments on DVE (fp16 -> 2x mode), overlap the MLP ---
            sums = sb.tile([64, 2], f32, tag="sums")
            dve_order = [
                nc.vector.reduce_sum(out=sums[:, 0:1], in_=y, axis=AX.X)
            ]
            ysq = sb.tile([64, L], f16, tag="ysq")
            dve_order.append(
                nc.vector.scalar_tensor_tensor(
                    out=ysq, in0=y, scalar=1.0, in1=y,
                    op0=OP.mult, op1=OP.mult, accum_out=sums[:, 1:2],
                )
            )
            eyb = sb.tile([64, 2], f32, tag="eyb")
            dve_order.append(
                nc.vector.tensor_scalar_mul(out=eyb, in0=sums, scalar1=1.0 / L)
            )
            ey = eyb[:, 0:1]
            ey2 = eyb[:, 1:2]
            yd2 = sb.tile([64, L], f16, tag="yd2")
            dve_order.append(
                nc.vector.tensor_scalar(
                    out=yd2, in0=y, scalar1=ey, scalar2=2.0,
                    op0=OP.subtract, op1=OP.mult,
                )
            )
            A = sb.tile([64, L], f16, tag="A")
            dve_order.append(
                nc.vector.tensor_scalar(
                    out=A, in0=ysq, scalar1=ey2, scalar2=None, op0=OP.subtract
                )
            )

            # --- MLP, full-L ---
            h_lv = ps.tile([128, L], f32, tag="hlv")
            h_mu = ps.tile([128, L], f32, tag="hmu")
            pe_order.append(
                nc.tensor.matmul(h_lv, w1lvT_a, xa, start=True, stop=False)
            )
            pe_order.append(
                nc.tensor.matmul(h_lv, w1lvT_b, xb, start=False, stop=True)
            )
            pe_order.append(
                nc.tensor.matmul(h_mu, w1muT_a, xa, start=True, stop=False)
            )
            pe_order.append(
                nc.tensor.matmul(h_mu, w1muT_b, xb, start=False, stop=True)
            )
            h_lv_s = sb.tile([128, L], bf16, tag="hlvs")
            act_order = [
                nc.scalar.activation(
                    out=h_lv_s, in_=h_lv, func=AF.Relu, bias=b1lv, scale=1.0
                )
            ]
            h_mu_s = sb.tile([128, L], bf16, tag="hmus")
            act_order.append(
                nc.scalar.activation(
                    out=h_mu_s, in_=h_mu, func=AF.Relu, bias=b1mu, scale=1.0
                )
            )
            lv_nb = ps.tile([64, L], f32, tag="lvnb")
            pe_order.append(
                nc.tensor.matmul(lv_nb, w2lvT, h_lv_s, start=True, stop=True)
            )
            mu_nb = ps.tile([64, L], f32, tag="munb")
            pe_order.append(
                nc.tensor.matmul(mu_nb, w2muT, h_mu_s, start=True, stop=True)
            )
            # lv tail on ACT: tanh -> exp
            t1 = sb.tile([64, L], f16, tag="t1")
            act_order.append(
                nc.scalar.activation(
                    out=t1, in_=lv_nb, func=AF.Tanh, bias=b2lv, scale=1.0
                )
            )
            iv = sb.tile([64, L], f16, tag="iv")
            act_order.append(
                nc.scalar.activation(out=iv, in_=t1, func=AF.Exp, scale=-1.0)
            )
            # mu tail on DVE: m2 = (mu_nb + b2mu)*yd2 ; m3 = A - m2 ; f = m3*iv
            m2 = sb.tile([64, L], f16, tag="m2")
            dve_order.append(
                nc.vector.scalar_tensor_tensor(
                    out=m2, in0=mu_nb, scalar=b2mu, in1=yd2,
                    op0=OP.add, op1=OP.mult,
                )
            )
            m3 = sb.tile([64, L], f16, tag="m3")
            dve_order.append(
                nc.vector.scalar_tensor_tensor(
                    out=m3, in0=m2, scalar=-1.0, in1=A,
                    op0=OP.mult, op1=OP.add,
                )
            )
            accT = sb.tile([64, 1], f32, tag="accT")
            scr = sb.tile([64, L], f16, tag="scr")
            dve_order.append(
                nc.vector.scalar_tensor_tensor(
                    out=scr, in0=m3, scalar=1.0, in1=iv,
                    op0=OP.mult, op1=OP.mult, accum_out=accT,
                )
            )
            # collapse over d, then 4B store straight from PSUM
            acc_ps = ps.tile([1, 1], f32, tag="accps")
            pe_order.append(
                nc.tensor.matmul(acc_ps, accT, ones, start=True, stop=True)
            )
            acc_sb = sb.tile([1, 1], f32, tag="accsb")
            nc.vector.tensor_copy(acc_sb, acc_ps)
            nc.sync.dma_start(out=acc_d[:, :], in_=acc_sb, single_packet=True)

            for order in (pe_order, act_order, dve_order):
                for a, b in zip(order[1:], order[:-1]):
                    add_dep_helper(a.ins, b.ins, sync=False, reason="stream-order")

    nc.compile()
    return nc


def pack_inputs(inputs: dict) -> list[dict]:
    import ml_dtypes

    bf = ml_dtypes.bfloat16
    x = np.asarray(inputs["x_samples"], dtype=np.float32)
    y = np.ascontiguousarray(np.asarray(inputs["y_samples"], dtype=np.float32))
    mu_W1 = np.asarray(inputs["mu_W1"], dtype=np.float32)
    mu_b1 = np.asarray(inputs["mu_b1"], dtype=np.float32)
    mu_W2 = np.asarray(inputs["mu_W2"], dtype=np.float32)
    mu_b2 = np.asarray(inputs["mu_b2"], dtype=np.float32)
    lv_W1 = np.asarray(inputs["lv_W1"], dtype=np.float32)
    lv_b1 = np.asarray(inputs["lv_b1"], dtype=np.float32)
    lv_W2 = np.asarray(inputs["lv_W2"], dtype=np.float32)
    lv_b2 = np.asarray(inputs["lv_b2"], dtype=np.float32)

    wi = np.zeros((128, WIC), bf)
    w1muT = mu_W1.T  # [192, 128]
    w1lvT = lv_W1.T
    wi[:, 0:128] = w1lvT[0:128].astype(bf)
    wi[:, 128:256] = w1muT[0:128].astype(bf)
    wi[0:64, 256:384] = w1lvT[128:192].astype(bf)
    wi[0:64, 384:512] = w1muT[128:192].astype(bf)
    wi[:, 512:576] = lv_W2.T.astype(bf)
    wi[:, 576:640] = mu_W2.T.astype(bf)
    b1 = np.zeros((128, 4), np.float32)
    b1[:, 0] = mu_b1
    b1[:, 1] = lv_b1
    b1[0:64, 2] = mu_b2
    b1[0:64, 3] = lv_b2

    xb16 = x.astype(bf)
    y16 = y.astype(np.float16)
    in_maps = []
    for b in range(NCORES):
        in_maps.append(
            {
                "wi": wi,
                "xa": np.ascontiguousarray(xb16[b, 0:128]),
                "xb": np.ascontiguousarray(xb16[b, 128:192]),
                "yb": np.ascontiguousarray(y16[b]),
                "b1": b1,
            }
        )
    return in_maps


def _combine(results) -> float:
    tot = 0.0
    for r in results:
        tot += float(r["acc"][0, 0])
    return tot


def kernel(**inputs) -> np.ndarray:
    from concourse.bass_utils import run_bass_kernel_spmd

    if "nc" not in _CACHE:
        _CACHE["nc"] = build_nc(debug=False)
    nc = _CACHE["nc"]

    in_maps = pack_inputs(inputs)
    res = run_bass_kernel_spmd(nc, in_maps, core_ids=list(range(NCORES)))
    loss = -0.5 * _combine(res.results) / (B * L)
    return np.array(loss, dtype=np.float32)


# revision 21
# speedup vs baseline: 1.1480x; 1.1480x over previous
"""CLUB loss kernel for 8x TRN2 NeuronCores.

Math: per sample b (L=512 positions, D=64 dims):
  mu     = MLP_mu(x);  logvar = tanh(MLP_lv(x));  iv = exp(-logvar)
  loss = mean over (b,l) of sum_d (positive - negative)
The pairwise LxL mean collapses via moments of y:
  pos - neg = -(0.5*iv) * (y^2 - Ey2 - mu*yd2),  yd2 = 2*(y - Ey)
so per core:  acc = sum_{d,l} (A - mu*yd2) * iv,  A = y^2 - Ey2,
and the host computes  loss = -0.5 * sum_b acc_b / (B*L).

Schedule (v4, built from perfetto traces):
 - ~13.2us of the measured window is fixed framework overhead (bass
   preamble + NRT's load-time epilogue that resets all 253 semaphores
   one EVENT_SEMAPHORE at a time + barrier butterflies). Body work sits
   on top of that.
 - input DMA placement: wi alone on the scalar HWDGE ring (sharing it
   serialized the completion sem ~2.3us late in v3); xa -> yb -> b1 on
   the sync HWDGE ring; xb on SWDGE (first in queue). HWDGE data is
   consumable ~3.2us after desc-gen (SDMA start + transfer + receipt).
 - PE HAM warmup: 6 K=1 matmuls on a memset [1,640] tile keep the PE
   busy from queue-start until the real matmuls begin, with ~zero SBUF
   read traffic (v2's K=128 dummies at ~450GB/s starved the input-DMA
   writes). HAM flips 1.2->2.4GHz after ~3.4us of sustained activity.
 - ACT runs relu_lv, relu_mu, tanh, exp (full-L each, (N+352)/1.2 ns);
   DVE runs the y-moments and the m2 -> m3 -> f tail; y and all
   elementwise intermediates are fp16 for the DVE 2x perf mode.
 - tail: f accumulates sum_l m3*iv per d; a [64,1]x[64,1] matmul
   collapses d; 4-byte single-packet store.

Precision: matmul operands bf16 (fp32 PE mode is 4x slower and fp32r
truncates to ~bf16 anyway); y/intermediates fp16 (loss error vs fp32
reference measured ~3e-4 with fp32, fp16 adds ~5e-4-level noise, well
under the 2e-2 gate); biases and accumulators fp32.
"""

import sys

if "/opt/trn_rl_repo" not in sys.path:
    sys.path.insert(0, "/opt/trn_rl_repo")

import numpy as np

B, L = 8, 512
XD, YD, H = 192, 64, 128
NCORES = 8
WIC = 640  # w1 pack (512) + w2 pack (128)
NDUMMY = 6  # K=1 N=512 HAM-warmup matmuls, ~427ns each cold

_CACHE: dict = {}


def build_nc(debug: bool = False):
    import concourse.bass as bass
    import concourse.bacc as bacc
    import concourse.tile as tile
    from concourse import mybir
    from concourse.tile import add_dep_helper

    f32 = mybir.dt.float32
    f16 = mybir.dt.float16
    bf16 = mybir.dt.bfloat16
    AF = mybir.ActivationFunctionType
    OP = mybir.AluOpType
    AX = mybir.AxisListType

    nc = bacc.Bacc("TRN2", target_bir_lowering=False, debug=debug)

    wi_d = nc.dram_tensor("wi", [128, WIC], bf16, kind="ExternalInput")
    xa_d = nc.dram_tensor("xa", [128, L], bf16, kind="ExternalInput")
    xb_d = nc.dram_tensor("xb", [64, L], bf16, kind="ExternalInput")
    yb_d = nc.dram_tensor("yb", [64, L], f16, kind="ExternalInput")
    b1_d = nc.dram_tensor("b1", [128, 4], f32, kind="ExternalInput")
    acc_d = nc.dram_tensor("acc", [1, 1], f32, kind="ExternalOutput")

    with tile.TileContext(nc) as tc:
        with (
            tc.tile_pool(name="sb", bufs=1) as sb,
            tc.tile_pool(name="ps", bufs=1, space=bass.MemorySpace.PSUM) as ps,
        ):
            # --- PE HAM warmup: K=1 matmuls on memset data (no SBUF load)
            warm = sb.tile([1, WIC], bf16, tag="warm")
            mset_warm = nc.gpsimd.memset(warm, 0.0)
            ones = sb.tile([64, 1], f32, tag="ones")
            nc.gpsimd.memset(ones, 1.0)
            dummy_ps = ps.tile([128, L], f32, tag="dummy")
            pe_order = []
            for i in range(NDUMMY):
                pe_order.append(
                    nc.tensor.matmul(
                        dummy_ps, warm[0:1, 0:128], warm[0:1, 0:512],
                        start=True, stop=True,
                    )
                )

            # --- input DMAs ---
            xat = sb.tile([128, L], bf16, tag="xat")
            d_xa = nc.sync.dma_start(out=xat, in_=xa_d[:, :])
            ybt = sb.tile([64, L], f16, tag="ybt")
            d_yb = nc.sync.dma_start(out=ybt, in_=yb_d[:, :])
            b1t = sb.tile([128, 4], f32, tag="b1t")
            d_b1 = nc.sync.dma_start(out=b1t, in_=b1_d[:, :])
            add_dep_helper(d_yb.ins, d_xa.ins, sync=False, reason="ring-order")
            add_dep_helper(d_b1.ins, d_yb.ins, sync=False, reason="ring-order")
            wit = sb.tile([128, WIC], bf16, tag="wit")
            nc.scalar.dma_start(out=wit, in_=wi_d[:, :])
            xbr = sb.tile([128, L], bf16, tag="xbr")
            d_xb = nc.gpsimd.dma_start(out=xbr[64:128, :], in_=xb_d[:, :])
            add_dep_helper(d_xb.ins, mset_warm.ins, sync=False, reason="warm-first")

            w1lvT_a = wit[:, 0:128]
            w1muT_a = wit[:, 128:256]
            w1lvT_b = wit[64:128, 256:384]
            w1muT_b = wit[64:128, 384:512]
            w2lvT = wit[:, 512:576]
            w2muT = wit[:, 576:640]
            b1mu = b1t[:, 0:1]
            b1lv = b1t[:, 1:2]
            b2mu = b1t[0:64, 2:3]
            b2lv = b1t[0:64, 3:4]
            xa = xat[:, :]
            xb = xbr[64:128, :]
            y = ybt[:, :]

            # --- y moments on DVE (fp16 -> 2x mode), overlap the MLP ---
            sums = sb.tile([64, 2], f32, tag="sums")
            dve_order = [
                nc.vector.reduce_sum(out=sums[:, 0:1], in_=y, axis=AX.X)
            ]
            ysq = sb.tile([64, L], f16, tag="ysq")
            dve_order.append(
                nc.vector.scalar_tensor_tensor(
                    out=ysq, in0=y, scalar=1.0, in1=y,
                    op0=OP.mult, op1=OP.mult, accum_out=sums[:, 1:2],
                )
            )
            eyb = sb.tile([64, 2], f32, tag="eyb")
            dve_order.append(
                nc.vector.tensor_scalar_mul(out=eyb, in0=sums, scalar1=1.0 / L)
            )
            ey = eyb[:, 0:1]
            ey2 = eyb[:, 1:2]
            yd2 = sb.tile([64, L], f16, tag="yd2")
            dve_order.append(
                nc.vector.tensor_scalar(
                    out=yd2, in0=y, scalar1=ey, scalar2=2.0,
                    op0=OP.subtract, op1=OP.mult,
                )
            )
            A = sb.tile([64, L], f16, tag="A")
            dve_order.append(
                nc.vector.tensor_scalar(
                    out=A, in0=ysq, scalar1=ey2, scalar2=None, op0=OP.subtract
                )
            )

            # --- MLP, full-L ---
            h_lv = ps.tile([128, L], f32, tag="hlv")
            h_mu = ps.tile([128, L], f32, tag="hmu")
            pe_order.append(
                nc.tensor.matmul(h_lv, w1lvT_a, xa, start=True, stop=False)
            )
            pe_order.append(
                nc.tensor.matmul(h_mu, w1muT_a, xa, start=True, stop=False)
            )
            pe_order.append(
                nc.tensor.matmul(h_lv, w1lvT_b, xb, start=False, stop=True)
            )
            pe_order.append(
                nc.tensor.matmul(h_mu, w1muT_b, xb, start=False, stop=True)
            )
            h_lv_s = sb.tile([128, L], bf16, tag="hlvs")
            act_order = [
                nc.scalar.activation(
                    out=h_lv_s, in_=h_lv, func=AF.Relu, bias=b1lv, scale=1.0
                )
            ]
            h_mu_s = sb.tile([128, L], bf16, tag="hmus")
            act_order.append(
                nc.scalar.activation(
                    out=h_mu_s, in_=h_mu, func=AF.Relu, bias=b1mu, scale=1.0
                )
            )
            lv_nb = ps.tile([64, L], f32, tag="lvnb")
            pe_order.append(
                nc.tensor.matmul(lv_nb, w2lvT, h_lv_s, start=True, stop=True)
            )
            mu_nb = ps.tile([64, L], f32, tag="munb")
            pe_order.append(
                nc.tensor.matmul(mu_nb, w2muT, h_mu_s, start=True, stop=True)
            )
            # lv tail on ACT: tanh -> exp
            t1 = sb.tile([64, L], f16, tag="t1")
            act_order.append(
                nc.scalar.activation(
                    out=t1, in_=lv_nb, func=AF.Tanh, bias=b2lv, scale=1.0
                )
            )
            iv = sb.tile([64, L], f16, tag="iv")
            act_order.append(
                nc.scalar.activation(out=iv, in_=t1, func=AF.Exp, scale=-1.0)
            )
            # mu tail on DVE: m2 = (mu_nb + b2mu)*yd2 ; m3 = A - m2 ; f = m3*iv
            m2 = sb.tile([64, L], f16, tag="m2")
            dve_order.append(
                nc.vector.scalar_tensor_tensor(
                    out=m2, in0=mu_nb, scalar=b2mu, in1=yd2,
                    op0=OP.add, op1=OP.mult,
                )
            )
            m3 = sb.tile([64, L], f16, tag="m3")
            dve_order.append(
                nc.vector.scalar_tensor_tensor(
                    out=m3, in0=m2, scalar=-1.0, in1=A,
                    op0=OP.mult, op1=OP.add,
                )
            )
            accT = sb.tile([64, 1], f32, tag="accT")
            scr = sb.tile([64, L], f16, tag="scr")
            dve_order.append(
                nc.vector.scalar_tensor_tensor(
                    out=scr, in0=m3, scalar=1.0, in1=iv,
                    op0=OP.mult, op1=OP.mult, accum_out=accT,
                )
            )
            # collapse over d, then 4B store straight from PSUM
            acc_ps = ps.tile([1, 1], f32, tag="accps")
            pe_order.append(
                nc.tensor.matmul(acc_ps, accT, ones, start=True, stop=True)
            )
            acc_sb = sb.tile([1, 1], f32, tag="accsb")
            nc.vector.tensor_copy(acc_sb, acc_ps)
            nc.sync.dma_start(out=acc_d[:, :], in_=acc_sb, single_packet=True)

            for order in (pe_order, act_order, dve_order):
                for a, b in zip(order[1:], order[:-1]):
                    add_dep_helper(a.ins, b.ins, sync=False, reason="stream-order")

    nc.compile()
    return nc


def pack_inputs(inputs: dict) -> list[dict]:
    import ml_dtypes

    bf = ml_dtypes.bfloat16
    x = np.asarray(inputs["x_samples"], dtype=np.float32)
    y = np.ascontiguousarray(np.asarray(inputs["y_samples"], dtype=np.float32))
    mu_W1 = np.asarray(inputs["mu_W1"], dtype=np.float32)
    mu_b1 = np.asarray(inputs["mu_b1"], dtype=np.float32)
    mu_W2 = np.asarray(inputs["mu_W2"], dtype=np.float32)
    mu_b2 = np.asarray(inputs["mu_b2"], dtype=np.float32)
    lv_W1 = np.asarray(inputs["lv_W1"], dtype=np.float32)
    lv_b1 = np.asarray(inputs["lv_b1"], dtype=np.float32)
    lv_W2 = np.asarray(inputs["lv_W2"], dtype=np.float32)
    lv_b2 = np.asarray(inputs["lv_b2"], dtype=np.float32)

    wi = np.zeros((128, WIC), bf)
    w1muT = mu_W1.T  # [192, 128]
    w1lvT = lv_W1.T
    wi[:, 0:128] = w1lvT[0:128].astype(bf)
    wi[:, 128:256] = w1muT[0:128].astype(bf)
    wi[64:128, 256:384] = w1lvT[128:192].astype(bf)
    wi[64:128, 384:512] = w1muT[128:192].astype(bf)
    wi[:, 512:576] = lv_W2.T.astype(bf)
    wi[:, 576:640] = mu_W2.T.astype(bf)
    b1 = np.zeros((128, 4), np.float32)
    b1[:, 0] = mu_b1
    b1[:, 1] = lv_b1
    b1[0:64, 2] = mu_b2
    b1[0:64, 3] = lv_b2

    xb16 = x.astype(bf)
    y16 = y.astype(np.float16)
    in_maps = []
    for b in range(NCORES):
        in_maps.append(
            {
                "wi": wi,
                "xa": np.ascontiguousarray(xb16[b, 0:128]),
                "xb": np.ascontiguousarray(xb16[b, 128:192]),
                "yb": np.ascontiguousarray(y16[b]),
                "b1": b1,
            }
        )
    return in_maps


def _combine(results) -> float:
    tot = 0.0
    for r in results:
        tot += float(r["acc"][0, 0])
    return tot


def kernel(**inputs) -> np.ndarray:
    from concourse.bass_utils import run_bass_kernel_spmd

    if "nc" not in _CACHE:
        _CACHE["nc"] = build_nc(debug=False)
    nc = _CACHE["nc"]

    in_maps = pack_inputs(inputs)
    res = run_bass_kernel_spmd(nc, in_maps, core_ids=list(range(NCORES)))
    loss = -0.5 * _combine(res.results) / (B * L)
    return np.array(loss, dtype=np.float32)
